# revision 15
# baseline (speedup 1.0000x reference)
"""GATv2 layer (N=1024, IN=OUT=128, H=4, D=32) on 8 Trainium2 NeuronCores.

Sharding: row-block of the output/adjacency (128 rows of i per core);
node features (pre-transposed h^T) and the projection weights are replicated.

Math per core (rows i of this core's block), with leakyrelu(x) = x - 0.8*min(x,0)
= 0.6*x + 0.4*|x| and sl[i,h] = a.Wlh[i,h,:] (cancels in the softmax over j),
sr[j,h] = a.Wrh[j,h,:]:

  e[i,j,h] = c*sr[j,h] + m_i[:,j] @ blockdiag(s*a)  - 100*(1-adj[i,j])

where per i-row either m_i = min(Wrh^T + Wlh_i, 0) produced on DVE
(tensor_scalar add+min, c=1, s=-0.8) or m_i = |Wrh^T + Wlh_i| produced on ACT
(Abs with per-partition bias, c=0.6, s=0.4).  Scores are O(3) so no
max-subtraction is needed, and the -100 mask term underflows exp to exact 0.

v2 layout: PSUM bank g accumulates scores for i-GROUP g (16 i's x all 1024 j),
columns (jt, il, h) = jt*64 + il*4 + h.  Each bank closes as soon as its 16
rows are done, so the exp for group g runs on ACT *during* stage 1 instead of
serializing in the tail.  6 rotating score banks + 2 V-projection banks let
the V projection/evacuation also overlap stage 1.  exp output goes to wT with
columns (jt, g, il, h) so stage-3 lhsT is a simple stride-4 slice per (jt,h).
LayerNorm rstd = exp(-0.5*ln(var+eps)) on ACT (same activation table set).
"""
import numpy as np
import ml_dtypes

import concourse.bacc as bacc
import concourse.tile as tile
from concourse import mybir
from concourse.bass_utils import run_bass_kernel_spmd

N = 1024
IN_DIM = 128
OUT_DIM = 128
H = 4
D = 32
NCORES = 8
BLK = N // NCORES  # 128 rows of i per core
NJT = 8            # j tiles of 128
NG = 8             # i groups of 16
GSZ = BLK // NG    # 16
F32 = mybir.dt.float32
F32R = mybir.dt.float32r
BF16 = mybir.dt.bfloat16
I32 = mybir.dt.int32
AF = mybir.ActivationFunctionType
ALU = mybir.AluOpType

# ACT (abs-variant) rows: ~26 of 128, spread inside each group of 16
_ACT_IL = {3, 8, 13}
_ACT_EXTRA = {(0, 5), (4, 5)}  # two groups get a 4th ACT row


def _on_act(i):
    g, il = divmod(i, GSZ)
    return il in _ACT_IL or (g, il) in _ACT_EXTRA


def build_program(apply_affine=True):
    nc = bacc.Bacc(trn_type="TRN2", target_bir_lowering=False, debug=False,
                   num_devices=NCORES)

    def din(name, shape, dt):
        return nc.dram_tensor(name, shape, dt, kind="ExternalInput").ap()

    # packed critical f32 inputs, two stages so the first matmul starts early
    critA_d = din("critA", [128, 2 * OUT_DIM + BLK + 512], F32R)  # wr|wl|hblkT|hT0
    critB_d = din("critB", [128, 512], F32R)                      # hT1
    critb_d = din("critb", [128, 3 * H], BF16)       # Adve | Aact | A1
    # adj folded per group: adj2[il*8+jt, g*128+p] = adj[16g+il, jt*128+p]
    adj_d = din("adj2", [128, N], I32)
    miscf_d = din("miscf", [128, OUT_DIM * 3], F32)  # W_v | gbc | bbc
    repc_d = din("rep_c", [H, H * BLK], BF16)        # c_i * I4 per i-column
    # SEL[il*8+jt, jt*64+il*4+h] = 1: expands folded mask rows into bank cols
    sel_d = din("SEL", [128, NJT * GSZ * H], BF16)
    y_d = nc.dram_tensor("y", [BLK, OUT_DIM], F32, kind="ExternalOutput").ap()

    with tile.TileContext(nc) as tc:
        with tc.tile_pool(name="keep", bufs=1) as keep, \
             tc.tile_pool(name="small", bufs=4) as small:
            # --- loads: staged packed DMAs on the critical path ---
            critA_sb = keep.tile([128, 2 * OUT_DIM + BLK + 512], F32R)
            nc.sync.dma_start(out=critA_sb, in_=critA_d)
            critB_sb = keep.tile([128, 512], F32R)
            nc.sync.dma_start(out=critB_sb, in_=critB_d)
            critb_sb = keep.tile([128, 3 * H], BF16)
            nc.sync.dma_start(out=critb_sb, in_=critb_d)
            wr_sb = critA_sb[:, 0:OUT_DIM]                      # f32r
            wl_sb = critA_sb[:, OUT_DIM:2 * OUT_DIM].bitcast(F32)
            hblkT_sb = critA_sb[:, 2 * OUT_DIM:2 * OUT_DIM + BLK].bitcast(F32)
            hT0_sb = critA_sb[:, 2 * OUT_DIM + BLK:]            # f32r
            hT1_sb = critB_sb                                   # f32r
            adve_sb = critb_sb[:, 0:H]
            aact_sb = critb_sb[:, H:2 * H]
            a1_sb = critb_sb[:, 2 * H:3 * H]
            # gpsimd queue: PE-warmup memset first, then bulk DMAs
            warm_sb = keep.tile([128, 64], BF16)
            nc.gpsimd.memset(warm_sb, 0.5)
            adj_sb = keep.tile([BLK, N], I32)
            nc.gpsimd.dma_start(out=adj_sb, in_=adj_d)
            miscf_sb = keep.tile([128, OUT_DIM * 3], F32)
            nc.gpsimd.dma_start(out=miscf_sb, in_=miscf_d)
            wv_sb = miscf_sb[:, 0:OUT_DIM]
            gbc_sb = miscf_sb[:, OUT_DIM:2 * OUT_DIM]
            bbc_sb = miscf_sb[:, 2 * OUT_DIM:3 * OUT_DIM]
            repc_sb = keep.tile([H, H * BLK], BF16)
            nc.gpsimd.dma_start(out=repc_sb, in_=repc_d)
            sel_sb = keep.tile([128, NJT * GSZ * H], BF16)
            nc.gpsimd.dma_start(out=sel_sb, in_=sel_d)

            wrhT_sb = keep.tile([128, N], BF16)       # (h@W_r)^T  [hd, j]
            wlhT_sb = keep.tile([128, BLK], F32)      # (hblk@W_l)^T [hd, i]
            vext_sb = keep.tile([128, NJT * (D + 1) * H], BF16)  # V + ones cols
            srT_sb = keep.tile([H, N], BF16)          # sr^T [h, j]
            maskb_sb = keep.tile([128, N], BF16)      # (adj2-1)*100, folded
            # exp scores, cols (jt, g, il, h)
            wT_sb = keep.tile([128, NJT * H * BLK], BF16)
            wT_r = wT_sb.rearrange("p (jt g c) -> p jt g c",
                                   jt=NJT, g=NG, c=GSZ * H)
            agg_sb = keep.tile([BLK, OUT_DIM], F32)

            with tc.tile_pool(name="ps0", bufs=2, space="PSUM") as ps0:
                # PE warmup: keep HAM busy so stage-0 matmuls run at 2.4 GHz
                wp0 = ps0.tile([128, 64], F32, tag="warm", bufs=1)
                for _ in range(60):
                    nc.tensor.matmul(wp0[0:64, :], warm_sb, warm_sb,
                                     start=True, stop=True,
                                     skip_group_check=True)
                # WrhT = W_r^T @ h^T  -> bf16 (gates stage 1); fp32r streams
                # 1 col/cycle and its input rounding is far above bf16
                bigs = []
                for half, hTh in ((0, hT0_sb), (1, hT1_sb)):
                    big = ps0.tile([128, 512], F32, tag="big")
                    nc.tensor.matmul(big, wr_sb, hTh,
                                     start=True, stop=True)
                    bigs.append(big)
                # evacuate both halves in parallel (DVE + ACT)
                nc.vector.tensor_copy(wrhT_sb[:, 0:512], bigs[0])
                nc.scalar.copy(wrhT_sb[:, 512:1024], bigs[1])
                # WlhT (this block), kept f32 for scalar/bias use
                wp = ps0.tile([128, 128], F32, tag="tp", bufs=1)
                nc.tensor.matmul(wp, wl_sb, hblkT_sb, start=True, stop=True)
                nc.vector.tensor_copy(wlhT_sb, wp)
                # srT = a^T . WrhT per head
                for half in range(2):
                    sp = ps0.tile([H, 512], F32, tag="sr", bufs=1)
                    nc.tensor.matmul(sp, a1_sb,
                                     wrhT_sb[:, half * 512:(half + 1) * 512],
                                     start=True, stop=True)
                    nc.scalar.copy(srT_sb[:, half * 512:(half + 1) * 512], sp)
            nc.gpsimd.memset(vext_sb, 1.0)

            # ------- stage 1: pairwise scores, per-group banks + inline exp ----
            with tc.tile_pool(name="ps1", bufs=6, space="PSUM") as ps1, \
                 tc.tile_pool(name="abs", bufs=20) as absp_pool:
                banks = [None] * NG
                pend_exp = []

                def do_exp(g):
                    nc.scalar.activation(wT_r[:, :, g], banks[g], AF.Exp)

                # mask from folded adj; gates only on the adj DMA
                nc.vector.tensor_scalar(maskb_sb, adj_sb, 1.0, 100.0,
                                        ALU.subtract, ALU.mult)
                for g in range(NG):
                    banks[g] = ps1.tile([128, NJT * GSZ * H], F32,
                                        name=f"bank{g}", tag="bank", bufs=6)
                    # bank opener: single full-bank start=True matmul (the
                    # start flag clears has_written for the WHOLE bank, so
                    # there must be exactly one, first)
                    nc.tensor.matmul(banks[g], maskb_sb[:, g * 128:(g + 1) * 128],
                                     sel_sb, start=True, stop=False,
                                     skip_group_check=True)
                    for il in range(GSZ):
                        i = g * GSZ + il
                        absp = absp_pool.tile([128, N], BF16, tag="absp")
                        if _on_act(i):
                            # |WrhT + wl_i|
                            nc.scalar.activation(absp, wrhT_sb, AF.Abs,
                                                 bias=wlhT_sb[:, i:i + 1],
                                                 scale=1.0)
                            arhs = aact_sb
                        else:
                            # min(WrhT + wl_i, 0)
                            nc.vector.tensor_scalar(absp, wrhT_sb,
                                                    wlhT_sb[:, i:i + 1],
                                                    0.0, ALU.add, ALU.min)
                            arhs = adve_sb
                        for jt in range(NJT):
                            nc.tensor.matmul(
                                banks[g][:, jt * 64 + il * H:
                                         jt * 64 + il * H + H],
                                absp[:, jt * 128:(jt + 1) * 128], arhs,
                                start=False, stop=False, skip_group_check=True)
                        if il == 5 and pend_exp:
                            do_exp(pend_exp.pop(0))
                        if il == 7:
                            # V projection for j-tile g, overlapped in stage 1
                            hTs = (hT0_sb[:, g * 128:(g + 1) * 128] if g < 4
                                   else hT1_sb[:, (g - 4) * 128:(g - 3) * 128]
                                   ).bitcast(F32)
                            vp = ps1.tile([128, 128], F32, tag="vp", bufs=2)
                            nc.tensor.matmul(vp, hTs, wv_sb, start=True,
                                             stop=True)
                            base = g * (D + 1) * H
                            dst = vext_sb[:, base:base + (D + 1) * H].rearrange(
                                "p (h dd) -> p h dd", h=H)[:, :, 0:D]
                            src = vp.rearrange("p (h dd) -> p h dd", h=H)
                            nc.scalar.copy(dst, src)
                    # close bank g with the sr term
                    for jt in range(NJT):
                        nc.tensor.matmul(banks[g][:, jt * 64:(jt + 1) * 64],
                                         srT_sb[:, jt * 128:(jt + 1) * 128],
                                         repc_sb[:, g * 64:(g + 1) * 64],
                                         start=False, stop=True,
                                         skip_group_check=True)
                    pend_exp.append(g)
                while pend_exp:
                    do_exp(pend_exp.pop(0))

            # ------------- stage 3: aggregate -------------
            with tc.tile_pool(name="ps3", bufs=4, space="PSUM") as ps3:
                accs = [ps3.tile([BLK, D + 1], F32, name=f"acc{hh}", tag="acc")
                        for hh in range(H)]
                for jt in range(NJT):
                    for hh in range(H):
                        lhsT = wT_sb[:, jt * 512 + hh:(jt + 1) * 512:H].opt()
                        rhs = vext_sb[:, jt * (D + 1) * H + hh * (D + 1):
                                      jt * (D + 1) * H + (hh + 1) * (D + 1)]
                        nc.tensor.matmul(accs[hh], lhsT, rhs,
                                         start=(jt == 0), stop=(jt == NJT - 1),
                                         skip_group_check=True)
                for hh in range(H):
                    rinv = small.tile([BLK, 1], F32, tag="rinv")
                    nc.vector.reciprocal(rinv, accs[hh][:, D:D + 1])
                    nc.vector.tensor_scalar_mul(
                        agg_sb[:, hh * D:(hh + 1) * D], accs[hh][:, 0:D], rinv)

            # ---------------- stage 4: LayerNorm + ReLU ----------------
            stats = small.tile([BLK, 6], F32, tag="stats")
            nc.vector.bn_stats(out=stats, in_=agg_sb)
            mv = small.tile([BLK, 2], F32, tag="mv")
            nc.vector.bn_aggr(out=mv, in_=stats)

            # rstd = 1/sqrt(var+eps): Quake initial guess + 2 Newton steps (DVE)
            veps = small.tile([BLK, 1], F32, tag="veps")
            nc.vector.tensor_scalar_add(veps, mv[:, 1:2], 1e-5)
            rstd = small.tile([BLK, 1], F32, tag="rstd")
            nc.vector.tensor_scalar(rstd.bitcast(I32), veps.bitcast(I32), 1,
                                    None, ALU.arith_shift_right)
            nc.vector.tensor_scalar(rstd.bitcast(I32), rstd.bitcast(I32), -1,
                                    0x5f3759df, ALU.mult, ALU.add)
            hv = small.tile([BLK, 1], F32, tag="hv")
            nc.vector.tensor_scalar_mul(hv, veps, -0.5)
            for _ in range(2):
                yy = small.tile([BLK, 1], F32, tag="yy")
                nc.vector.tensor_tensor(yy, rstd, rstd, ALU.mult)
                nc.vector.tensor_scalar(yy, yy, hv, 1.5, ALU.mult, ALU.add)
                nc.vector.tensor_tensor(rstd, rstd, yy, ALU.mult)
            nmr = small.tile([BLK, 1], F32, tag="nmr")
            nc.vector.tensor_tensor(nmr, mv[:, 0:1], rstd, ALU.mult)
            nc.vector.tensor_scalar_mul(nmr, nmr, -1.0)
            yt = keep.tile([BLK, OUT_DIM], F32)
            nc.vector.tensor_scalar(yt, agg_sb, rstd, nmr, ALU.mult, ALU.add)
            if apply_affine:
                nc.vector.tensor_tensor(yt, yt, gbc_sb, ALU.mult)
                nc.vector.tensor_tensor(yt, yt, bbc_sb, ALU.add)
            nc.vector.tensor_scalar_max(yt, yt, 0.0)
            nc.sync.dma_start(out=y_d, in_=yt)

    nc.compile()
    return nc


_NC = {}


def _get_program(apply_affine):
    if apply_affine not in _NC:
        _NC[apply_affine] = build_program(apply_affine)
    return _NC[apply_affine]


def _consts(a, ln_g, ln_b):
    bf = ml_dtypes.bfloat16
    a = np.asarray(a, np.float32)
    Adve = np.zeros((128, H), np.float32)
    Aact = np.zeros((128, H), np.float32)
    A1 = np.zeros((128, H), np.float32)
    for hh in range(H):
        Adve[hh * D:(hh + 1) * D, hh] = -0.8 * a
        Aact[hh * D:(hh + 1) * D, hh] = 0.4 * a
        A1[hh * D:(hh + 1) * D, hh] = a
    # per-i sr coefficient: 0.6 for ACT(|.|) rows, 1.0 for DVE(min) rows
    rep_c = np.zeros((H, H * BLK), np.float32)
    for i in range(BLK):
        c = 0.6 if _on_act(i) else 1.0
        rep_c[:, H * i:H * i + H] = c * np.eye(H, dtype=np.float32)
    SEL = np.zeros((128, NJT * GSZ * H), np.float32)
    for il in range(GSZ):
        for jt in range(NJT):
            for hh in range(H):
                SEL[il * NJT + jt, jt * 64 + il * H + hh] = 1.0
    return {
        "Adve": Adve.astype(bf), "Aact": Aact.astype(bf), "A1": A1.astype(bf),
        "rep_c": rep_c.astype(bf), "SEL": SEL.astype(bf),
        "gbc": np.tile(np.asarray(ln_g, np.float32)[None, :], (BLK, 1)),
        "bbc": np.tile(np.asarray(ln_b, np.float32)[None, :], (BLK, 1)),
    }


def kernel(h, adj, W_l, W_r, W_v, a, ln_g, ln_b, _trace=False, _tmpdir=None):
    affine = not (np.all(np.asarray(ln_g) == 1.0) and np.all(np.asarray(ln_b) == 0.0))
    nc = _get_program(affine)
    h = np.asarray(h, np.float32)
    hT = np.ascontiguousarray(h.T)
    adj = np.ascontiguousarray(np.asarray(adj, np.int32))
    consts = _consts(a, ln_g, ln_b)
    W_l = np.asarray(W_l, np.float32)
    W_r = np.asarray(W_r, np.float32)
    W_v = np.asarray(W_v, np.float32)
    critb = np.concatenate([consts["Adve"], consts["Aact"], consts["A1"]], axis=1)
    miscf = np.ascontiguousarray(
        np.concatenate([W_v, consts["gbc"], consts["bbc"]], axis=1))
    base = {
        "critb": np.ascontiguousarray(critb),
        "miscf": miscf,
        "rep_c": consts["rep_c"],
        "SEL": consts["SEL"],
    }
    base["critB"] = np.ascontiguousarray(hT[:, 512:])
    in_maps = []
    for c in range(NCORES):
        m = dict(base)
        m["critA"] = np.ascontiguousarray(np.concatenate(
            [W_r, W_l, hT[:, c * BLK:(c + 1) * BLK], hT[:, :512]], axis=1))
        # fold: adj2[il*8+jt, g*128+p] = adj[16g+il, jt*128+p]
        m["adj2"] = np.ascontiguousarray(
            adj[c * BLK:(c + 1) * BLK].reshape(NG, GSZ, NJT, 128)
            .transpose(1, 2, 0, 3).reshape(128, N))
        in_maps.append(m)
    kw = {}
    if _trace:
        kw = dict(trace=True, tmpdir=_tmpdir)
    res = run_bass_kernel_spmd(nc, in_maps, list(range(NCORES)), **kw)
    y = np.concatenate([res.results[c]["y"] for c in range(NCORES)], axis=0)
    if _trace:
        return y, res
    return y


# revision 17
# speedup vs baseline: 1.0979x; 1.0979x over previous
"""GATv2 layer (N=1024, IN=OUT=128, H=4, D=32) on 8 Trainium2 NeuronCores.

Sharding: row-block of the output/adjacency (128 rows of i per core);
node features (pre-transposed h^T) and the projection weights are replicated.

Math per core (rows i of this core's block), with leakyrelu(x) = x - 0.8*min(x,0)
= 0.6*x + 0.4*|x| and sl[i,h] = a.Wlh[i,h,:] (cancels in the softmax over j),
sr[j,h] = a.Wrh[j,h,:]:

  e[i,j,h] = c*sr[j,h] + m_i[:,j] @ blockdiag(s*a)  - 100*(1-adj[i,j])

where per i-row either m_i = min(Wrh^T + Wlh_i, 0) produced on DVE
(tensor_scalar add+min, c=1, s=-0.8) or m_i = |Wrh^T + Wlh_i| produced on ACT
(Abs with per-partition bias, c=0.6, s=0.4).  Scores are O(3) so no
max-subtraction is needed, and the -100 mask term underflows exp to exact 0.

v2 layout: PSUM bank g accumulates scores for i-GROUP g (16 i's x all 1024 j),
columns (jt, il, h) = jt*64 + il*4 + h.  Each bank closes as soon as its 16
rows are done, so the exp for group g runs on ACT *during* stage 1 instead of
serializing in the tail.  6 rotating score banks + 2 V-projection banks let
the V projection/evacuation also overlap stage 1.  exp output goes to wT with
columns (jt, g, il, h) so stage-3 lhsT is a simple stride-4 slice per (jt,h).
LayerNorm rstd = exp(-0.5*ln(var+eps)) on ACT (same activation table set).
"""
import numpy as np
import ml_dtypes

import concourse.bacc as bacc
import concourse.tile as tile
from concourse import mybir
from concourse.bass_utils import run_bass_kernel_spmd

N = 1024
IN_DIM = 128
OUT_DIM = 128
H = 4
D = 32
NCORES = 8
BLK = N // NCORES  # 128 rows of i per core
NJT = 8            # j tiles of 128
NG = 8             # i groups of 16
GSZ = BLK // NG    # 16
F32 = mybir.dt.float32
F32R = mybir.dt.float32r
BF16 = mybir.dt.bfloat16
I32 = mybir.dt.int32
AF = mybir.ActivationFunctionType
ALU = mybir.AluOpType

# ACT (abs-variant) rows: ~29 of 128, spread inside each group of 16
_ACT_IL = {3, 8, 13}
_ACT_EXTRA = {(0, 6), (2, 6), (4, 6), (6, 6), (7, 6)}


def _on_act(i):
    g, il = divmod(i, GSZ)
    return il in _ACT_IL or (g, il) in _ACT_EXTRA


def build_program(apply_affine=True):
    nc = bacc.Bacc(trn_type="TRN2", target_bir_lowering=False, debug=False,
                   num_devices=NCORES)

    def din(name, shape, dt):
        return nc.dram_tensor(name, shape, dt, kind="ExternalInput").ap()

    # packed critical bf16 inputs, two stages so the first matmul starts early
    critA_d = din("critA", [128, 2 * OUT_DIM + BLK + 512], BF16)  # wr|wl|hblkT|hT0
    critB_d = din("critB", [128, 512], BF16)                      # hT1
    critb_d = din("critb", [128, 3 * H], BF16)       # Adve | Aact | A1
    # mask folded per group, precomputed on host:
    # maskb[il*8+jt, g*128+p] = 100*(adj[16g+il, jt*128+p] - 1)
    maskb_d = din("maskb", [128, N], BF16)
    wv_d = din("wv", [128, OUT_DIM], BF16)           # W_v
    if apply_affine:
        gb_d = din("gbbc", [128, 2 * OUT_DIM], F32)  # gbc | bbc
    repc_d = din("rep_c", [H, H * BLK], BF16)        # c_i * I4 per i-column
    # SEL[il*8+jt, jt*64+il*4+h] = 1: expands folded mask rows into bank cols
    sel_d = din("SEL", [128, NJT * GSZ * H], BF16)
    y_d = nc.dram_tensor("y", [BLK, OUT_DIM], F32, kind="ExternalOutput").ap()

    with tile.TileContext(nc) as tc:
        with tc.tile_pool(name="keep", bufs=1) as keep, \
             tc.tile_pool(name="small", bufs=4) as small:
            # --- loads: staged packed DMAs on the critical path ---
            critA_sb = keep.tile([128, 2 * OUT_DIM + BLK + 512], BF16)
            nc.sync.dma_start(out=critA_sb, in_=critA_d)
            critB_sb = keep.tile([128, 512], BF16)
            nc.sync.dma_start(out=critB_sb, in_=critB_d)
            critb_sb = keep.tile([128, 3 * H], BF16)
            nc.sync.dma_start(out=critb_sb, in_=critb_d)
            wr_sb = critA_sb[:, 0:OUT_DIM]
            wl_sb = critA_sb[:, OUT_DIM:2 * OUT_DIM]
            hblkT_sb = critA_sb[:, 2 * OUT_DIM:2 * OUT_DIM + BLK]
            hT0_sb = critA_sb[:, 2 * OUT_DIM + BLK:]
            hT1_sb = critB_sb
            adve_sb = critb_sb[:, 0:H]
            aact_sb = critb_sb[:, H:2 * H]
            a1_sb = critb_sb[:, 2 * H:3 * H]
            # gpsimd queue: PE-warmup memset, then bulk DMAs in need-order
            warm_sb = keep.tile([128, 64], BF16)
            nc.gpsimd.memset(warm_sb, 0.5)
            maskb_sb = keep.tile([128, N], BF16)
            nc.gpsimd.dma_start(out=maskb_sb, in_=maskb_d)
            sel_sb = keep.tile([128, NJT * GSZ * H], BF16)
            nc.gpsimd.dma_start(out=sel_sb, in_=sel_d)
            repc_sb = keep.tile([H, H * BLK], BF16)
            nc.gpsimd.dma_start(out=repc_sb, in_=repc_d)
            wv_sb = keep.tile([128, OUT_DIM], BF16)
            nc.gpsimd.dma_start(out=wv_sb, in_=wv_d)
            if apply_affine:
                gb_sb = keep.tile([128, 2 * OUT_DIM], F32)
                nc.gpsimd.dma_start(out=gb_sb, in_=gb_d)
                gbc_sb = gb_sb[:, 0:OUT_DIM]
                bbc_sb = gb_sb[:, OUT_DIM:2 * OUT_DIM]

            wrhT_sb = keep.tile([128, N], BF16)       # (h@W_r)^T  [hd, j]
            wlhT_sb = keep.tile([128, BLK], F32)      # (hblk@W_l)^T [hd, i]
            vext_sb = keep.tile([128, NJT * (D + 1) * H], BF16)  # V + ones cols
            srT_sb = keep.tile([H, N], BF16)          # sr^T [h, j]
            # exp scores, cols (jt, g, il, h)
            wT_sb = keep.tile([128, NJT * H * BLK], BF16)
            wT_r = wT_sb.rearrange("p (jt g c) -> p jt g c",
                                   jt=NJT, g=NG, c=GSZ * H)
            agg_sb = keep.tile([BLK, OUT_DIM], F32)

            with tc.tile_pool(name="ps0", bufs=2, space="PSUM") as ps0:
                # PE warmup: keep HAM busy so stage-0 matmuls run at 2.4 GHz
                wp0 = ps0.tile([128, 64], F32, tag="warm", bufs=1)
                for _ in range(60):
                    nc.tensor.matmul(wp0[0:64, :], warm_sb, warm_sb,
                                     start=True, stop=True,
                                     skip_group_check=True)
                # WrhT = W_r^T @ h^T  -> bf16 (gates stage 1); fp32r streams
                # 1 col/cycle and its input rounding is far above bf16
                bigs = []
                for half, hTh in ((0, hT0_sb), (1, hT1_sb)):
                    big = ps0.tile([128, 512], F32, tag="big")
                    nc.tensor.matmul(big, wr_sb, hTh,
                                     start=True, stop=True)
                    bigs.append(big)
                # evacuate both halves in parallel (DVE + ACT)
                nc.vector.tensor_copy(wrhT_sb[:, 0:512], bigs[0])
                nc.scalar.copy(wrhT_sb[:, 512:1024], bigs[1])
                # WlhT (this block), kept f32 for scalar/bias use
                wp = ps0.tile([128, 128], F32, tag="tp", bufs=1)
                nc.tensor.matmul(wp, wl_sb, hblkT_sb, start=True, stop=True)
                nc.vector.tensor_copy(wlhT_sb, wp)
                # srT = a^T . WrhT per head
                for half in range(2):
                    sp = ps0.tile([H, 512], F32, tag="sr", bufs=1)
                    nc.tensor.matmul(sp, a1_sb,
                                     wrhT_sb[:, half * 512:(half + 1) * 512],
                                     start=True, stop=True)
                    nc.scalar.copy(srT_sb[:, half * 512:(half + 1) * 512], sp)
            nc.gpsimd.memset(vext_sb, 1.0)

            # ------- stage 1: pairwise scores, per-group banks + inline exp ----
            with tc.tile_pool(name="ps1", bufs=6, space="PSUM") as ps1, \
                 tc.tile_pool(name="abs", bufs=20) as absp_pool:
                banks = [None] * NG
                pend_exp = []

                def do_exp(g):
                    nc.scalar.activation(wT_r[:, :, g], banks[g], AF.Exp)

                for g in range(NG):
                    banks[g] = ps1.tile([128, NJT * GSZ * H], F32,
                                        name=f"bank{g}", tag="bank", bufs=6)
                    # bank opener: single full-bank start=True matmul (the
                    # start flag clears has_written for the WHOLE bank, so
                    # there must be exactly one, first)
                    nc.tensor.matmul(banks[g], maskb_sb[:, g * 128:(g + 1) * 128],
                                     sel_sb, start=True, stop=False,
                                     skip_group_check=True)
                    for il in range(GSZ):
                        i = g * GSZ + il
                        absp = absp_pool.tile([128, N], BF16, tag="absp")
                        if _on_act(i):
                            # |WrhT + wl_i|
                            nc.scalar.activation(absp, wrhT_sb, AF.Abs,
                                                 bias=wlhT_sb[:, i:i + 1],
                                                 scale=1.0)
                            arhs = aact_sb
                        else:
                            # min(WrhT + wl_i, 0)
                            nc.vector.tensor_scalar(absp, wrhT_sb,
                                                    wlhT_sb[:, i:i + 1],
                                                    0.0, ALU.add, ALU.min)
                            arhs = adve_sb
                        for jt in range(NJT):
                            nc.tensor.matmul(
                                banks[g][:, jt * 64 + il * H:
                                         jt * 64 + il * H + H],
                                absp[:, jt * 128:(jt + 1) * 128], arhs,
                                start=False, stop=False, skip_group_check=True)
                        if il == 14 and pend_exp:
                            do_exp(pend_exp.pop(0))
                        if il == 7:
                            # V projection for j-tile g, overlapped in stage 1
                            hTs = (hT0_sb[:, g * 128:(g + 1) * 128] if g < 4
                                   else hT1_sb[:, (g - 4) * 128:(g - 3) * 128])
                            vp = ps1.tile([128, 128], F32, tag="vp", bufs=2)
                            nc.tensor.matmul(vp, hTs, wv_sb, start=True,
                                             stop=True)
                            base = g * (D + 1) * H
                            dst = vext_sb[:, base:base + (D + 1) * H].rearrange(
                                "p (h dd) -> p h dd", h=H)[:, :, 0:D]
                            src = vp.rearrange("p (h dd) -> p h dd", h=H)
                            nc.scalar.copy(dst, src)
                    # close bank g with the sr term
                    for jt in range(NJT):
                        nc.tensor.matmul(banks[g][:, jt * 64:(jt + 1) * 64],
                                         srT_sb[:, jt * 128:(jt + 1) * 128],
                                         repc_sb[:, g * 64:(g + 1) * 64],
                                         start=False, stop=True,
                                         skip_group_check=True)
                    pend_exp.append(g)
                while pend_exp:
                    do_exp(pend_exp.pop(0))

            # ------------- stage 3: aggregate -------------
            with tc.tile_pool(name="ps3", bufs=4, space="PSUM") as ps3:
                accs = [ps3.tile([BLK, D + 1], F32, name=f"acc{hh}", tag="acc")
                        for hh in range(H)]
                for jt in range(NJT):
                    for hh in range(H):
                        lhsT = wT_sb[:, jt * 512 + hh:(jt + 1) * 512:H].opt()
                        rhs = vext_sb[:, jt * (D + 1) * H + hh * (D + 1):
                                      jt * (D + 1) * H + (hh + 1) * (D + 1)]
                        nc.tensor.matmul(accs[hh], lhsT, rhs,
                                         start=(jt == 0), stop=(jt == NJT - 1),
                                         skip_group_check=True)
                for hh in range(H):
                    rinv = small.tile([BLK, 1], F32, tag="rinv")
                    nc.vector.reciprocal(rinv, accs[hh][:, D:D + 1])
                    nc.vector.tensor_scalar_mul(
                        agg_sb[:, hh * D:(hh + 1) * D], accs[hh][:, 0:D], rinv)

            # ---------------- stage 4: LayerNorm + ReLU ----------------
            stats = small.tile([BLK, 6], F32, tag="stats")
            nc.vector.bn_stats(out=stats, in_=agg_sb)
            mv = small.tile([BLK, 2], F32, tag="mv")
            nc.vector.bn_aggr(out=mv, in_=stats)

            # rstd = 1/sqrt(var+eps): Quake initial guess + 2 Newton steps (DVE)
            veps = small.tile([BLK, 1], F32, tag="veps")
            nc.vector.tensor_scalar_add(veps, mv[:, 1:2], 1e-5)
            rstd = small.tile([BLK, 1], F32, tag="rstd")
            nc.vector.tensor_scalar(rstd.bitcast(I32), veps.bitcast(I32), 1,
                                    None, ALU.arith_shift_right)
            nc.vector.tensor_scalar(rstd.bitcast(I32), rstd.bitcast(I32), -1,
                                    0x5f3759df, ALU.mult, ALU.add)
            hv = small.tile([BLK, 1], F32, tag="hv")
            nc.vector.tensor_scalar_mul(hv, veps, -0.5)
            for _ in range(2):
                yy = small.tile([BLK, 1], F32, tag="yy")
                nc.vector.tensor_tensor(yy, rstd, rstd, ALU.mult)
                nc.vector.tensor_scalar(yy, yy, hv, 1.5, ALU.mult, ALU.add)
                nc.vector.tensor_tensor(rstd, rstd, yy, ALU.mult)
            nmr = small.tile([BLK, 1], F32, tag="nmr")
            nc.vector.tensor_tensor(nmr, mv[:, 0:1], rstd, ALU.mult)
            nc.vector.tensor_scalar_mul(nmr, nmr, -1.0)
            yt = keep.tile([BLK, OUT_DIM], F32)
            nc.vector.tensor_scalar(yt, agg_sb, rstd, nmr, ALU.mult, ALU.add)
            if apply_affine:
                nc.vector.tensor_tensor(yt, yt, gbc_sb, ALU.mult)
                nc.vector.tensor_tensor(yt, yt, bbc_sb, ALU.add)
            nc.vector.tensor_scalar_max(yt, yt, 0.0)
            nc.sync.dma_start(out=y_d, in_=yt)

    nc.compile()
    return nc


_NC = {}


def _get_program(apply_affine):
    if apply_affine not in _NC:
        _NC[apply_affine] = build_program(apply_affine)
    return _NC[apply_affine]


def _consts(a, ln_g, ln_b):
    bf = ml_dtypes.bfloat16
    a = np.asarray(a, np.float32)
    Adve = np.zeros((128, H), np.float32)
    Aact = np.zeros((128, H), np.float32)
    A1 = np.zeros((128, H), np.float32)
    for hh in range(H):
        Adve[hh * D:(hh + 1) * D, hh] = -0.8 * a
        Aact[hh * D:(hh + 1) * D, hh] = 0.4 * a
        A1[hh * D:(hh + 1) * D, hh] = a
    # per-i sr coefficient: 0.6 for ACT(|.|) rows, 1.0 for DVE(min) rows
    rep_c = np.zeros((H, H * BLK), np.float32)
    for i in range(BLK):
        c = 0.6 if _on_act(i) else 1.0
        rep_c[:, H * i:H * i + H] = c * np.eye(H, dtype=np.float32)
    SEL = np.zeros((128, NJT * GSZ * H), np.float32)
    for il in range(GSZ):
        for jt in range(NJT):
            for hh in range(H):
                SEL[il * NJT + jt, jt * 64 + il * H + hh] = 1.0
    return {
        "Adve": Adve.astype(bf), "Aact": Aact.astype(bf), "A1": A1.astype(bf),
        "rep_c": rep_c.astype(bf), "SEL": SEL.astype(bf),
        "gbc": np.tile(np.asarray(ln_g, np.float32)[None, :], (BLK, 1)),
        "bbc": np.tile(np.asarray(ln_b, np.float32)[None, :], (BLK, 1)),
    }


def kernel(h, adj, W_l, W_r, W_v, a, ln_g, ln_b, _trace=False, _tmpdir=None):
    bf = ml_dtypes.bfloat16
    affine = not (np.all(np.asarray(ln_g) == 1.0) and np.all(np.asarray(ln_b) == 0.0))
    nc = _get_program(affine)
    h = np.asarray(h, np.float32)
    hT = np.ascontiguousarray(h.T).astype(bf)
    adj = np.ascontiguousarray(np.asarray(adj, np.int32))
    consts = _consts(a, ln_g, ln_b)
    W_l = np.asarray(W_l, np.float32).astype(bf)
    W_r = np.asarray(W_r, np.float32).astype(bf)
    W_v = np.asarray(W_v, np.float32).astype(bf)
    critb = np.concatenate([consts["Adve"], consts["Aact"], consts["A1"]], axis=1)
    base = {
        "critb": np.ascontiguousarray(critb),
        "wv": np.ascontiguousarray(W_v),
        "rep_c": consts["rep_c"],
        "SEL": consts["SEL"],
    }
    if affine:
        base["gbbc"] = np.ascontiguousarray(
            np.concatenate([consts["gbc"], consts["bbc"]], axis=1))
    base["critB"] = np.ascontiguousarray(hT[:, 512:])
    maskf = 100.0 * (adj.astype(np.float32) - 1.0)
    in_maps = []
    for c in range(NCORES):
        m = dict(base)
        m["critA"] = np.ascontiguousarray(np.concatenate(
            [W_r, W_l, hT[:, c * BLK:(c + 1) * BLK], hT[:, :512]], axis=1))
        # fold: maskb[il*8+jt, g*128+p] = 100*(adj[16g+il, jt*128+p]-1)
        m["maskb"] = np.ascontiguousarray(
            maskf[c * BLK:(c + 1) * BLK].reshape(NG, GSZ, NJT, 128)
            .transpose(1, 2, 0, 3).reshape(128, N).astype(bf))
        in_maps.append(m)
    kw = {}
    if _trace:
        kw = dict(trace=True, tmpdir=_tmpdir)
    res = run_bass_kernel_spmd(nc, in_maps, list(range(NCORES)), **kw)
    y = np.concatenate([res.results[c]["y"] for c in range(NCORES)], axis=0)
    if _trace:
        return y, res
    return y
